# revision 4
# baseline (speedup 1.0000x reference)
"""AutoregressiveRAM kernel for trn2 — 8-core sharded windowed one-hot design.

Global state: 8192 bits at (q in [0,128), ng in [0,64)); neuron b sits at
(q=b//64, ng=b%64); core c owns neurons with ng in [8c, 8c+8) (slot s=ng%8).
Placement is conflict-aware: bits co-tapped by any neuron never share a
16-bit word (word (g, ng) = bits at partitions 16g..16g+15, col ng).

Per core, per step:
  bits (local XOR layout) --(DVE mask-mult)--> bitsrep bf16 [128, C]
  --(PE pack matmul, 2^(q%16) weights, gated cols)--> psum words (replicated
    across partitions; col cl holds word (g(cl), cl%64), g per-column free)
  --(ACT cast + pos-word write)--> swords u16 [128, C+NPOS]
  --(DVE AND with per-(slot, col) masks; 4-lvl pairwise add tree over W=16)
  --> sel2 = 2^r * tapbit per (slot, window)
  --(DVE x POWC + reduce)--> addr f32 [128, 8] --(ACT cast)--> u16
  --(DVE onehot over 64 table words + onehot16 bit select)--> new bits u16
  --(GPSIMD: 7 relative remote_dma_broadcast sends, XOR peer layout)
  --(ACT u16->f32, SP DMA y row slice out)

Exchange: local block k of the bits array holds core (me XOR k)'s slice;
own block is k=0 (cols 0..7) on every core, so the program is SPMD-static.
"""
import sys
sys.path.insert(0, '/opt/trn_rl_repo')
import numpy as np

BITS = 8192
NB_T = 10
POS = 4
P = 128
S = 8
NCORES = 8
HI_W = 64
NSTEPS = 4096

# Probed exchange topology: a relative remote_dma with XOR-distance k lands on
# the core whose shard id is (mine ^ SLOT_MAP[k]); SLOT_MAP is an involution.
# Receiver slot k therefore holds shard (me ^ SLOT_MAP[k]); global block b sits
# at local slot SLOT_MAP[b ^ me] (SLOT_MAP is its own inverse).
SLOT_MAP = [0, 1, 2, 3, 6, 7, 4, 5]

_CACHE = {}


# ---------------------------------------------------------------- placement
def spread_placement(conn, seed=0):
    """Assign each state bit to a global position so that co-tapped bits never
    share a 16-bit word. Returns perm[8192] = position of bit b, where
    position pos = q*64 + ng (word id w = (q//16)*64 + ng, slot r = q%16).

    Equivalently: word_of[b] in [0,512), r_of[b] in [0,16)."""
    rng = np.random.default_rng(seed)
    nbr = [set() for _ in range(BITS)]
    for row in conn:
        bb = sorted({int(x) for x in row if x < BITS})
        for i in range(len(bb)):
            for j in range(i + 1, len(bb)):
                nbr[bb[i]].add(bb[j])
                nbr[bb[j]].add(bb[i])
    deg = np.array([len(x) for x in nbr])
    order = np.argsort(-deg, kind='stable')
    NW = 512
    members = [set() for _ in range(NW)]
    word_of = np.full(BITS, -1, np.int64)
    wlist = np.arange(NW)
    for b in order:
        bnb = nbr[b]
        # try words in random order, prefer emptier ones among first hits
        cand = rng.permutation(wlist)
        placed = False
        for w in cand:
            m = members[w]
            if len(m) >= 16:
                continue
            if m & bnb:
                continue
            m.add(int(b))
            word_of[b] = w
            placed = True
            break
        if not placed:
            return None
    r_of = np.full(BITS, -1, np.int64)
    for w in range(NW):
        for i, b in enumerate(sorted(members[w])):
            r_of[b] = i
    return word_of, r_of


def build_global_layout(conn, seed=0):
    """Returns pos_of[b] = q*64+ng for each bit b (the neuron->position map)."""
    res = spread_placement(conn, seed)
    assert res is not None, "spread placement failed"
    word_of, r_of = res
    g = word_of // 64
    ng = word_of % 64
    q = 16 * g + r_of
    pos_of = q * 64 + ng
    # sanity: bijection
    assert len(np.unique(pos_of)) == BITS
    return pos_of


# ------------------------------------------------------------- per-core color
def color_core(tapsets, J, W, NPOS, c, pos_of, rng):
    """Per-core window assignment via per-neuron bipartite matching with
    on-demand word replication.

    Each word (g, nl) gets one seeded column in its class; when a neuron's
    taps cannot be matched to distinct windows, a replica column of the
    blocking word is added (if the class has room).

    Returns None on failure else dict with colg [C], ANDM [P,S*C] u16,
    POSM [P,S*NPOS] u16, POWC [P,S*(J+NPOS)] f32."""
    C = J * W
    colg = np.full(C, -1, np.int64)
    ANDM = np.zeros((P, S * C), np.uint16)
    POSM = np.zeros((P, S * NPOS), np.uint16)
    POWC = np.zeros((P, S * (J + NPOS)), np.float64)
    JW64 = C // 64  # columns per n-class

    # word key = (g, nl) -> list of (j, cl) copies
    copies = {}
    free_cols = {}  # nl -> list of free cl
    for nl in range(64):
        cols = [(nl + 64 * t) % C for t in range(JW64)]
        rng.shuffle(cols)
        # seed 8 words at the first 8 columns
        for g in range(8):
            cl = cols[g]
            colg[cl] = g
            copies[(g, nl)] = [(cl // W, cl)]
        free_cols[nl] = cols[8:]

    def add_replica(g, nl, avoid):
        fl = free_cols[nl]
        best_i = -1
        for i, cl in enumerate(fl):
            if (cl // W) not in avoid:
                best_i = i
                break
        if best_i < 0 and fl:
            best_i = 0  # take a matched window; caller re-augments
        if best_i < 0:
            return None
        cl = fl.pop(best_i)
        colg[cl] = g
        copies[(g, nl)].append((cl // W, cl))
        return (cl // W, cl)

    for p in range(P):
        for s in range(S):
            taps = tapsets[p * S + s]
            state_taps = []
            npos_used = 0
            for bit, wsum in taps:
                if bit >= BITS:
                    i = bit - BITS
                    col = npos_used
                    npos_used += 1
                    if col >= NPOS:
                        return None
                    POSM[p, (s * NPOS) + col] |= np.uint16(1 << i)
                    POWC[p, s * (J + NPOS) + J + col] += wsum / float(2 ** i)
                else:
                    posn = pos_of[bit]
                    q = posn // 64
                    ng = posn % 64
                    nl = 8 * SLOT_MAP[(ng // 8) ^ c] + (ng % 8)
                    state_taps.append((q // 16, nl, q % 16, wsum))
            # bipartite matching taps -> windows (augmenting paths)
            match_w = {}   # window j -> tap index
            match_t = {}   # tap index -> (j, cl)
            for ti, (g, nl, r, wsum) in enumerate(state_taps):
                # try to find augmenting path from ti
                def try_assign(ti2, visited):
                    for (j, cl) in copies[(state_taps[ti2][0], state_taps[ti2][1])]:
                        if j in visited:
                            continue
                        visited.add(j)
                        if j not in match_w or try_assign(match_w[j], visited):
                            match_w[j] = ti2
                            match_t[ti2] = (j, cl)
                            return True
                    return False

                attempts = 0
                while not try_assign(ti, set()):
                    attempts += 1
                    if attempts > 30:
                        return None
                    rep = add_replica(g, nl, set(match_w.keys()))
                    if rep is None:
                        # class full: add a replica for some other word in the
                        # neuron to break the blocking chain
                        progress = False
                        for (g2, nl2, _, _) in state_taps:
                            if add_replica(g2, nl2, set(match_w.keys())) is not None:
                                progress = True
                                break
                        if not progress:
                            return None
            for ti, (g, nl, r, wsum) in enumerate(state_taps):
                j, cl = match_t[ti]
                ANDM[p, s * C + cl] |= np.uint16(1 << r)
                POWC[p, s * (J + NPOS) + j] += wsum / float(2 ** r)
    POWCf = POWC.astype(np.float32)
    assert np.array_equal(POWCf.astype(np.float64), POWC)
    return dict(colg=colg, ANDM=ANDM, POSM=POSM, POWC=POWCf)


def build_core_consts(c, conn, transition_memory, pos_of, J, W, NPOS, seed=1,
                      tries=4):
    import ml_dtypes
    inv = np.empty(BITS, np.int64)
    inv[pos_of] = np.arange(BITS)   # inv[pos] = neuron at that position
    # tapsets for this core's neurons, ordered by (p, s)
    tapsets = []
    for p in range(P):
        for s in range(S):
            b = inv[p * 64 + 8 * c + s]
            row = conn[b]
            agg = {}
            for k in range(NB_T):
                t = int(row[k])
                agg[t] = agg.get(t, 0.0) + float(2 ** (NB_T - 1 - k))
            tapsets.append(sorted(agg.items()))
    res = None
    for t_i in range(tries):
        rng = np.random.default_rng(seed + c + 1000 * t_i)
        res = color_core(tapsets, J, W, NPOS, c, pos_of, rng)
        if res is not None:
            break
    if res is None:
        return None
    C = J * W
    # MASKG bf16 [P, C]: gate [q//16 == colg[cl]] (0 where column unused)
    MASKG = (np.arange(P)[:, None] // 16 == res['colg'][None, :]).astype(np.float32)
    # PACKW [P, 128] bf16: lhsT[q, m] = 2^(q%16)
    PACKW = np.tile((2.0 ** (np.arange(P) % 16))[:, None], (1, P)).astype(np.float32)
    # TBLT u16 [P, 64*S] transposed (w outer, s inner), for this core's neurons
    T = transition_memory
    TBLT = np.zeros((P, HI_W, S), np.uint16)
    neuron_at = np.zeros((P, S), np.int64)
    for p in range(P):
        for s in range(S):
            b = inv[p * 64 + 8 * c + s]
            neuron_at[p, s] = b
    Tb = (T > 0.5).astype(np.uint16).reshape(BITS, HI_W, 16)
    U16 = (Tb << np.arange(16, dtype=np.uint16)[None, None, :]).sum(-1).astype(np.uint16)
    TBLT[:, :, :] = U16[neuron_at, :].transpose(0, 2, 1)
    IOTA_WT = np.tile(np.arange(HI_W, dtype=np.uint16)[None, :, None], (P, 1, S))
    IOTA16T = np.tile(np.arange(16, dtype=np.uint16)[None, :, None], (P, 1, S))
    POW2T = np.tile((np.uint16(1) << np.arange(16, dtype=np.uint16))[None, :, None],
                    (P, 1, S))
    # POSW u16 [P, 4096]: pos word for step t (index t-1); pos bit i at bitpos i
    t = np.arange(1, NSTEPS + 1)
    posw = np.zeros(NSTEPS, np.uint16)
    for i in range(POS):
        posw |= (((t >> (3 - i)) & 1) << i).astype(np.uint16)
    POSW = np.tile(posw[None, :], (P, 1))
    return dict(res, MASKG=MASKG, PACKW=PACKW, TBLT=TBLT, IOTA_WT=IOTA_WT,
                IOTA16T=IOTA16T, POW2T=POW2T, POSW=POSW, neuron_at=neuron_at)


# ------------------------------------------------------- numpy step emulation
def emulate_cores(consts_all, conn, pos_of, state0_bits, J, W, NPOS, T):
    """Pure-numpy emulation of the device program for T steps; returns
    out_bits [T, 8192] in neuron order. Used to validate the constants."""
    C = J * W
    inv = np.empty(BITS, np.int64)
    inv[pos_of] = np.arange(BITS)
    # local bits arrays per core (XOR layout), u16
    bits_loc = []
    stg = state0_bits[inv].reshape(P, 64)  # bit value at global (q, ng)
    for c in range(NCORES):
        loc = np.zeros((P, 64), np.uint16)
        for bl in range(8):
            sb_ = c ^ SLOT_MAP[bl]
            loc[:, 8 * bl:8 * bl + 8] = stg[:, 8 * sb_:8 * sb_ + 8]
        bits_loc.append(loc)
    outs = np.zeros((T, BITS), np.float32)
    for step in range(1, T + 1):
        posword = 0
        for i in range(POS):
            posword |= ((step >> (3 - i)) & 1) << i
        newblocks = []
        for c in range(NCORES):
            cc = consts_all[c]
            b = bits_loc[c]
            # pack: word at col cl = sum_r 2^r * b[16*g+ r, cl%64] for g=colg
            colg = cc['colg']
            swords = np.zeros((C + NPOS,), np.uint32)
            valid = colg >= 0
            gg = np.where(valid, colg, 0)
            nn = np.arange(C) % 64
            rows = (16 * gg[None, :] + np.arange(16)[:, None])  # [16, C]
            vals = b[rows, nn[None, :]].astype(np.uint32)
            w = (vals << np.arange(16, dtype=np.uint32)[:, None]).sum(0)
            swords[:C] = np.where(valid, w, 0)
            swords[C:] = posword
            sw = swords.astype(np.uint16)
            # AND + window reduce
            am = cc['ANDM'].reshape(P, S, C)
            tmp = am & sw[None, None, :C]
            sel = tmp.reshape(P, S, J, W).sum(-1)  # windows have <=1 nonzero
            pm = cc['POSM'].reshape(P, S, NPOS)
            selp = pm & sw[None, None, C:]
            sel2 = np.concatenate([sel, selp], axis=2).astype(np.float64)
            addr = (sel2 * cc['POWC'].reshape(P, S, J + NPOS)).sum(-1)
            addri = addr.astype(np.int64)
            assert (np.abs(addr - addri) < 1e-6).all()
            hi6 = addri >> 4
            lo4 = addri & 15
            tb = cc['TBLT'].transpose(0, 2, 1)  # [P, S, 64]
            word = np.take_along_axis(tb.astype(np.int64), hi6[:, :, None], 2)[:, :, 0]
            bit = (word >> lo4) & 1
            newblocks.append(bit.astype(np.uint16))  # [P, S] new own bits
            # record in neuron order
            outs[step - 1, cc['neuron_at'].reshape(-1)] = bit.reshape(-1)
        # exchange
        for c in range(NCORES):
            for k in range(NCORES):
                bits_loc[c][:, 8 * k:8 * k + 8] = newblocks[c ^ SLOT_MAP[k]]
    return outs


# ------------------------------------------------------------- device program
def build_program(J, W, NPOS, n_steps):
    from concourse import bacc, mybir
    import concourse.bass as bass_mod
    from contextlib import ExitStack
    C = J * W
    JP = J + NPOS
    nc = bacc.Bacc('TRN2', target_bir_lowering=False, debug=False)
    dt = mybir.dt

    def param(name, cols, dtype=dt.float32):
        return nc.declare_dram_parameter(name, [P, cols], dtype, isOutput=False)

    x_packw = param('x_packw', P // 2)
    x_maskg = param('x_maskg', C // 2)
    x_andm = param('x_andm', (S * C) // 2)
    x_posm = param('x_posm', (S * NPOS) // 2)
    x_powc = param('x_powc', S * JP)
    x_tblt = param('x_tblt', (HI_W * S) // 2)
    x_iotawt = param('x_iotawt', (HI_W * S) // 2)
    x_iota16 = param('x_iota16', (16 * S) // 2)
    x_pow2 = param('x_pow2', (16 * S) // 2)
    x_posw = param('x_posw', NSTEPS // 2)
    x_state0 = param('x_state0', 64 // 2)
    y = nc.declare_dram_parameter('y', [n_steps, P * S], dt.float32, isOutput=True)

    es = ExitStack()
    block = es.enter_context(nc.Block())
    sb = lambda name, cols, dtype: es.enter_context(
        nc.sbuf_tensor(name, [P, cols], dtype))
    PACKW = sb('PACKW', P, dt.bfloat16)
    MASKG = sb('MASKG', C, dt.bfloat16)
    ANDM = sb('ANDM', S * C, dt.uint16)
    POSM = sb('POSM', S * NPOS, dt.uint16)
    POWC = sb('POWC', S * JP, dt.float32)
    TBLT = sb('TBLT', HI_W * S, dt.uint16)
    IOTA_WT = sb('IOTA_WT', HI_W * S, dt.uint16)
    IOTA16 = sb('IOTA16', 16 * S, dt.uint16)
    POW2 = sb('POW2', 16 * S, dt.uint16)
    POSW = sb('POSW', NSTEPS, dt.uint16)
    bits2 = sb('bits2', 2 * 64, dt.uint16)      # [b=2, n=64] state double buffer
    bitsrep = sb('bitsrep', C, dt.bfloat16)
    swords = sb('swords', C + NPOS, dt.uint16)
    tmpA = sb('tmpA', S * C, dt.uint16)
    tmpB = sb('tmpB', S * J * (W // 2), dt.uint16)
    sel2 = sb('sel2', S * JP, dt.uint16)
    addrt = sb('addrt', S * JP, dt.float32)
    addr = sb('addr', S, dt.float32)
    addrI = sb('addrI', S, dt.uint16)
    hi6 = sb('hi6', S, dt.uint16)
    lo4 = sb('lo4', S, dt.uint16)
    oh = sb('oh', HI_W * S, dt.uint16)
    wtmp = sb('wtmp', HI_W * S, dt.uint16)
    word2 = sb('word2', S, dt.uint16)
    oh16 = sb('oh16', 16 * S, dt.uint16)
    ptmp = sb('ptmp', 16 * S, dt.uint16)
    pw = sb('pw', S, dt.uint16)
    tb = sb('tb', S, dt.uint16)
    bitf = sb('bitf', 2 * S, dt.float32)
    psum = es.enter_context(nc.psum_tensor('psum', [P, C], dt.float32))

    s_in = es.enter_context(nc.semaphore('s_in'))
    rsem = es.enter_context(nc.semaphore('rsem'))
    lsem = es.enter_context(nc.semaphore('lsem'))
    psem = es.enter_context(nc.semaphore('psem'))
    s_rhs = es.enter_context(nc.semaphore('s_rhs'))
    s_pe = es.enter_context(nc.semaphore('s_pe'))
    s_sw = es.enter_context(nc.semaphore('s_sw'))
    s_addr = es.enter_context(nc.semaphore('s_addr'))
    s_ai = es.enter_context(nc.semaphore('s_ai'))
    s_bits = es.enter_context(nc.semaphore('s_bits'))
    s_bf = es.enter_context(nc.semaphore('s_bf'))
    s_dma = es.enter_context(nc.semaphore('s_dma'))
    s_sent = es.enter_context(nc.semaphore('s_sent'))

    N_IN = 11

    AND = bass_mod.mybir.AluOpType.bitwise_and
    MULT = bass_mod.mybir.AluOpType.mult
    ADD = bass_mod.mybir.AluOpType.add
    EQ = bass_mod.mybir.AluOpType.is_equal
    X = bass_mod.mybir.AxisListType.X
    ds = bass_mod.ds
    mybir = bass_mod.mybir

    @block.sync
    def _(sync):
        for t_, src in [(PACKW, x_packw), (MASKG, x_maskg), (ANDM, x_andm),
                        (POSM, x_posm), (POWC, x_powc), (TBLT, x_tblt),
                        (IOTA_WT, x_iotawt), (IOTA16, x_iota16), (POW2, x_pow2),
                        (POSW, x_posw)]:
            sync.dma_start(out=t_[:], in_=src[:].bitcast(t_.dtype)).then_inc(s_in, 16)
        sync.dma_start(out=bits2[:, 0:64], in_=x_state0[:].bitcast(dt.uint16)
                       ).then_inc(s_in, 16)
        with sync.Fori(0, n_steps) as row:
            sync.wait_ge(s_bf, row + 1)
            off = sync.scalar_reg_alu(mybir.AluOpType.bitwise_and, row, 1)
            sync.dma_start(
                out=y[ds(row, 1), :],
                in_=bitf[:].rearrange('p (b s) -> p b s', b=2)[:, ds(off, 1), :],
            ).then_inc(s_dma, 16)

    @block.tensor
    def _(tensor):
        tensor.wait_ge(s_in, 16 * N_IN)
        with tensor.Fori(0, n_steps // 2) as m:
            for stp_off in (1, 2):
                sm = tensor.snap(m, min_val=0, max_val=n_steps // 2)
                t2 = tensor.scalar_reg_alu(mybir.AluOpType.mult, sm, 2)
                stp = tensor.scalar_reg_alu(mybir.AluOpType.add, t2, stp_off)
                tensor.wait_ge(s_rhs, stp)
                stpm1 = tensor.scalar_reg_alu(mybir.AluOpType.add, t2, stp_off - 1)
                tensor.wait_ge(s_sw, stpm1)
                tensor.matmul(psum[:, 0:512], PACKW[:], bitsrep[:, 0:512],
                              start=True, stop=True)
                tensor.matmul(psum[:, 512:C], PACKW[:], bitsrep[:, 512:C],
                              start=True, stop=True).then_inc(s_pe, 1)

    @block.scalar
    def _(scalar):
        scalar.wait_ge(s_in, 16 * N_IN)
        with scalar.Fori(0, n_steps // 2) as m:
            for half, stp_off in ((1, 1), (0, 2)):
                sm = scalar.snap(m, min_val=0, max_val=n_steps // 2)
                t2 = scalar.scalar_reg_alu(mybir.AluOpType.mult, sm, 2)
                stp = scalar.scalar_reg_alu(mybir.AluOpType.add, t2, stp_off)
                stpm1 = scalar.scalar_reg_alu(mybir.AluOpType.add, t2, stp_off - 1)
                scalar.wait_ge(s_pe, stp)
                scalar.copy(out=swords[:, 0:C], in_=psum[:, 0:C])
                scalar.copy(out=swords[:, C:C + NPOS],
                            in_=POSW[:, ds(stpm1, 1)].broadcast_to([P, NPOS])
                            ).then_inc(s_sw, 1)
                scalar.wait_ge(s_addr, stp)
                scalar.copy(out=addrI[:], in_=addr[:]).then_inc(s_ai, 1)
                scalar.wait_ge(s_bits, stp)
                dtgt = scalar.scalar_reg_alu(mybir.AluOpType.mult, stpm1, 16)
                scalar.wait_ge(s_dma, dtgt)
                # bitf half: rows alternate; step stp writes row stp-1 whose
                # parity is (stp-1)&1 = 1-half... step odd (half=1): row even
                bh = 0 if half == 1 else 1
                scalar.copy(
                    out=bitf[:, S * bh:S * bh + S],
                    in_=bits2[:, 64 * half:64 * half + S],
                ).then_inc(s_bf, 1)

    @block.vector
    def _(vector):
        _lp = nc.allow_low_precision(reason='u16 one-hot reductions, exact')
        _lp.__enter__()
        vector.wait_ge(s_in, 16 * N_IN)

        with vector.Fori(0, n_steps // 2) as m:
          for half, stp_off in ((1, 1), (0, 2)):
            sm = vector.snap(m, min_val=0, max_val=n_steps // 2)
            t2 = vector.scalar_reg_alu(mybir.AluOpType.mult, sm, 2)
            stp = vector.scalar_reg_alu(mybir.AluOpType.add, t2, stp_off)
            stpm1 = vector.scalar_reg_alu(mybir.AluOpType.add, t2, stp_off - 1)
            # bitsrep from state(stp-1) in buffer (1-half)
            rtgt = vector.scalar_reg_alu(mybir.AluOpType.mult, stpm1, 14)
            vector.wait_ge(rsem, rtgt)
            rbase = 64 * (1 - half)
            vector.tensor_tensor(
                bitsrep[:].rearrange('p (r n) -> p r n', n=64),
                bits2[:, rbase:rbase + 64].unsqueeze(1)
                .broadcast_to([P, C // 64, 64]),
                MASKG[:].rearrange('p (r n) -> p r n', n=64),
                MULT).then_inc(s_rhs, 1)
            vector.drain()
            vector.wait_ge(s_sw, stp)
            # AND scan
            vector.tensor_tensor(
                tmpA[:].rearrange('p (s c) -> p s c', s=S),
                swords[:, 0:C].unsqueeze(1).broadcast_to([P, S, C]),
                ANDM[:].rearrange('p (s c) -> p s c', s=S),
                AND)
            vector.drain()
            # window add tree: W -> W/2 -> ... -> 1, ping-pong tmpA/tmpB
            bufs = [(tmpA, W), (tmpB, W // 2)]
            cur = 0
            width = W
            while width > 1:
                hw = width // 2
                src, sstr = bufs[cur]
                vS = src[:].rearrange('p (s j w) -> p s j w', s=S, j=J, w=sstr)
                if hw == 1:
                    out = sel2[:].rearrange('p (s j) -> p s j', s=S)[
                        :, :, 0:J].unsqueeze(3)
                else:
                    dst, dstr = bufs[1 - cur]
                    out = dst[:].rearrange('p (s j w) -> p s j w', s=S, j=J,
                                           w=dstr)[:, :, :, 0:hw]
                vector.tensor_tensor(out, vS[:, :, :, 0:hw],
                                     vS[:, :, :, hw:width], ADD)
                vector.drain()
                cur = 1 - cur
                width = hw
            # pos taps
            vector.tensor_tensor(
                sel2[:].rearrange('p (s j) -> p s j', s=S)[:, :, J:JP],
                swords[:, C:C + NPOS].unsqueeze(1).broadcast_to([P, S, NPOS]),
                POSM[:].rearrange('p (s n) -> p s n', s=S),
                AND)
            vector.drain()
            # addr = sum sel2 * POWC
            vector.tensor_tensor(addrt[:], sel2[:], POWC[:], MULT)
            vector.drain()
            vector.tensor_reduce(addr[:].rearrange('p s -> p s'),
                                 addrt[:].rearrange('p (s j) -> p s j', s=S),
                                 axis=X, op=ADD).then_inc(s_addr, 1)
            vector.drain()
            vector.wait_ge(s_ai, stp)
            vector.tensor_scalar(hi6[:], addrI[:], 4, None,
                                 mybir.AluOpType.logical_shift_right)
            vector.tensor_scalar(lo4[:], addrI[:], 15, None, AND)
            vector.drain()
            # word one-hot (transposed: w outer, s inner)
            vector.tensor_tensor(oh[:].rearrange('p (w s) -> p w s', w=HI_W),
                                 hi6[:].unsqueeze(1).broadcast_to([P, HI_W, S]),
                                 IOTA_WT[:].rearrange('p (w s) -> p w s', w=HI_W),
                                 EQ)
            vector.drain()
            vector.tensor_tensor(wtmp[:].rearrange('p (w s) -> p w s', w=HI_W),
                                 oh[:].rearrange('p (w s) -> p w s', w=HI_W),
                                 TBLT[:].rearrange('p (w s) -> p w s', w=HI_W),
                                 MULT)
            vector.drain()
            vector.tensor_reduce(
                word2[:].rearrange('p s -> p s'),
                wtmp[:].rearrange('p (w s) -> p w s', w=HI_W).transpose([0, 2, 1]),
                axis=X, op=ADD)
            vector.drain()
            # pw = 1 << lo4
            vector.tensor_tensor(oh16[:].rearrange('p (b s) -> p b s', b=16),
                                 lo4[:].unsqueeze(1).broadcast_to([P, 16, S]),
                                 IOTA16[:].rearrange('p (b s) -> p b s', b=16),
                                 EQ)
            vector.drain()
            vector.tensor_tensor(ptmp[:].rearrange('p (b s) -> p b s', b=16),
                                 oh16[:].rearrange('p (b s) -> p b s', b=16),
                                 POW2[:].rearrange('p (b s) -> p b s', b=16),
                                 MULT)
            vector.drain()
            vector.tensor_reduce(
                pw[:].rearrange('p s -> p s'),
                ptmp[:].rearrange('p (b s) -> p b s', b=16).transpose([0, 2, 1]),
                axis=X, op=ADD)
            vector.drain()
            vector.tensor_tensor(tb[:], word2[:], pw[:], AND)
            vector.drain()
            # guard: wait for the previous step's sends to complete before
            # overwriting our block (slightly stronger than the stp-2
            # minimum, but always non-negative and off the critical path)
            lt2 = vector.scalar_reg_alu(mybir.AluOpType.mult, stpm1, 7 * 16)
            vector.wait_ge(lsem, lt2)
            vector.tensor_scalar(
                bits2[:, 64 * half:64 * half + S],
                tb[:], 0, None, mybir.AluOpType.is_gt).then_inc(s_bits, 1)
            vector.drain()
        _lp.__exit__(None, None, None)

    @block.gpsimd
    def _(g):
        g.wait_ge(s_in, 16 * N_IN)
        assert n_steps % 2 == 0
        with g.Fori(0, n_steps // 2) as m:
            for half, stp_off in ((1, 1), (0, 2)):   # step = 2m + stp_off
                sm = g.snap(m, min_val=0, max_val=n_steps // 2)
                t2 = g.scalar_reg_alu(mybir.AluOpType.mult, sm, 2)
                stp = g.scalar_reg_alu(mybir.AluOpType.add, t2, stp_off)
                g.wait_ge(s_bits, stp)
                rtgt = g.scalar_reg_alu(
                    mybir.AluOpType.mult,
                    g.scalar_reg_alu(mybir.AluOpType.add, t2, stp_off - 1), 14)
                g.wait_ge(rsem, rtgt)
                base = 64 * half
                src = bits2[:, base:base + S]
                for k in range(1, NCORES):
                    dst = bits2[:, base + S * k:base + S * k + S]
                    rdests = [(0, k) if i == k else None for i in range(NCORES)]
                    g.remote_dma_broadcast(out_ap=dst, in_ap=src, remote_sem=rsem,
                                           local_sem=lsem, rdests=rdests
                                           ).then_inc(psem, 1)
                ptgt = g.scalar_reg_alu(mybir.AluOpType.mult, stp, 7)
                g.wait_ge(psem, ptgt)
                g.trigger_dma(count=7)

    es.close()
    nc.finalize()
    return nc


# ------------------------------------------------------------------ packaging
def _f32c(u16arr):
    a = np.ascontiguousarray(u16arr.reshape(P, -1))
    assert a.shape[1] % 2 == 0
    return a.view(np.float32)


def _bf16c(f32arr):
    import ml_dtypes
    a = np.ascontiguousarray(f32arr.reshape(P, -1)).astype(ml_dtypes.bfloat16)
    assert a.shape[1] % 2 == 0
    return a.view(np.uint16).view(np.float32)


def host_step0(initial_memory, initial_connections):
    ic = np.asarray(initial_connections)
    im = np.asarray(initial_memory)
    pos0 = np.zeros(POS, np.int64)
    sel0 = pos0[ic]
    pw0 = 2 ** np.arange(3, -1, -1)
    addr0 = (sel0 * pw0).sum(1)
    out0 = im[np.arange(BITS), addr0].astype(np.float32)
    return out0


def build_all(conn, transition_memory, n_steps):
    """Build layout, per-core constants, and the program. Returns
    (prog, meta) where meta has everything needed to make in_maps."""
    for (J, W) in [(104, 8), (112, 8), (120, 8)]:
        pos_of = build_global_layout(conn, seed=0)
        # NPOS: max pos taps per neuron (count duplicates as distinct cols? we
        # fold duplicates, so count distinct pos bits per neuron)
        npos_need = 1
        for row in conn:
            pp = len({int(x) for x in row if x >= BITS})
            npos_need = max(npos_need, pp)
        NPOS = npos_need
        consts = []
        ok = True
        for c in range(NCORES):
            cc = build_core_consts(c, conn, transition_memory, pos_of, J, W, NPOS)
            if cc is None:
                ok = False
                break
            consts.append(cc)
        if ok:
            prog = build_program(J, W, NPOS, n_steps)
            return prog, dict(J=J, W=W, NPOS=NPOS, pos_of=pos_of, consts=consts,
                              n_steps=n_steps)
    raise RuntimeError('coloring failed for all (J, W) candidates')


def make_in_maps(meta, state0_bits):
    J, W, NPOS = meta['J'], meta['W'], meta['NPOS']
    pos_of = meta['pos_of']
    inv = np.empty(BITS, np.int64)
    inv[pos_of] = np.arange(BITS)
    stg = state0_bits[inv].reshape(P, 64).astype(np.uint16)
    in_maps = []
    for c in range(NCORES):
        cc = meta['consts'][c]
        loc = np.zeros((P, 64), np.uint16)
        for bl in range(8):
            sb_ = c ^ SLOT_MAP[bl]
            loc[:, 8 * bl:8 * bl + 8] = stg[:, 8 * sb_:8 * sb_ + 8]
        ins = {
            'x_packw': _bf16c(cc['PACKW']),
            'x_maskg': _bf16c(cc['MASKG']),
            'x_andm': _f32c(cc['ANDM']),
            'x_posm': _f32c(cc['POSM']),
            'x_powc': np.ascontiguousarray(cc['POWC'].reshape(P, -1)),
            'x_tblt': _f32c(cc['TBLT']),
            'x_iotawt': _f32c(cc['IOTA_WT']),
            'x_iota16': _f32c(cc['IOTA16T']),
            'x_pow2': _f32c(cc['POW2T']),
            'x_posw': _f32c(cc['POSW']),
            'x_state0': _f32c(loc),
        }
        in_maps.append(ins)
    return in_maps


def assemble_output(meta, results, out0, length):
    out = np.empty((length, BITS), np.float32)
    out[0] = out0
    n_steps = meta['n_steps']
    for c in range(NCORES):
        yc = results[c]['y']  # [n_steps, 1024]; row r = step r+1
        idx = meta['consts'][c]['neuron_at'].reshape(-1)
        nrows = min(length - 1, n_steps)
        out[1:1 + nrows, idx] = yc[0:nrows]
    return out


def kernel(transition_memory, initial_memory, transition_connections,
           initial_connections, length):
    from concourse.bass_utils import run_bass_kernel_spmd
    import os
    length = int(length)
    conn = np.asarray(transition_connections)
    key = 'prog'
    if key not in _CACHE:
        prog, meta = build_all(conn, np.asarray(transition_memory), NSTEPS)
        _CACHE[key] = (prog, meta)
    prog, meta = _CACHE[key]

    out0 = host_step0(initial_memory, initial_connections)
    state0_bits = (out0 > 0.5).astype(np.uint16)
    in_maps = make_in_maps(meta, state0_bits)
    import time
    t0 = time.time()
    res = run_bass_kernel_spmd(prog, in_maps, core_ids=list(range(NCORES)),
                               trace=bool(os.environ.get('KBENCH_TRACE')))
    global _LAST_RESULTS, _LAST_EXEC_WALL_NS
    _LAST_EXEC_WALL_NS = int((time.time() - t0) * 1e9)
    _LAST_RESULTS = res
    return assemble_output(meta, res.results, out0, length)
